# revision 1
# baseline (speedup 1.0000x reference)
"""Trainium2 Bass kernel for a 2-layer GCN (AblationGCN) on 8 NeuronCores.

Contract: kernel(**inputs) takes the FULL unsharded inputs of the reference
(x [100000,165] f32, edge_index [2,1600000] int, W1,b1,W2,b2,Wc,bc) and
returns the FULL output [100000, 2] f32.

Strategy (SPMD, one Bass program on cores 0-7):
  - dst-range sharding: core c owns output nodes [c*12500, (c+1)*12500).
  - L1 dense (replicated): h1 = x @ W1 for all nodes via TensorE with
    host-transposed x; g1 = dinv*h1 stored as a bf16 node-major DRAM table.
  - L1 aggregation: edges (+self loops) grouped by dst block (128 nodes),
    each block padded to a uniform TB tiles of 128 edges. Per tile:
    indirect-DMA gather of g1[src] rows, a one-hot selection matrix S
    (iota == dstlocal on VectorE), and a TensorE matmul S^T @ G
    accumulating the block in PSUM. Epilogue: *dinv, +bias, relu.
  - a1 blocks are PE-transposed and exchanged with one AllGather.
  - L2 dense from the allgathered feature-major a1T; L2 aggregation same
    as L1; classifier a2T.T @ Wc + bc per block.
Host preprocessing computes degrees/dinv and the per-core uniform tile
metadata (indices, local dst one-hot keys).
"""
import numpy as np

P = 128
N = 100000
N_CORES = 8
IN_F = 165
HID = 128
SHARD = N // N_CORES            # 12500
NB = (SHARD + P - 1) // P       # 98
NPAD = ((N + P - 1) // P) * P   # 100096
SHARD_PAD = NB * P              # 12544
NPAD2 = N_CORES * SHARD_PAD     # 100352
NT1 = NPAD // P                 # 782


def _split_excess_waits(nc, max_waits=1):
    """This walrus build only accepts one sync-wait command per instruction;
    hoist extras onto NoOps placed just before the carrying instruction."""
    import concourse.mybir as mybir
    for f in nc.m.functions:
        for b in f.blocks:
            insts = b.instructions
            new_list = []
            changed = False
            for ins in insts:
                si = ins.sync_info
                if si is not None and len(si.on_wait) > max_waits:
                    waits = list(si.on_wait)
                    keep = waits[:max_waits]
                    extra = waits[max_waits:]
                    for ci in range(0, len(extra), max_waits):
                        chunk = extra[ci:ci + max_waits]
                        nop = mybir.InstNoOp(name=f"{ins.name}_wsplit{ci}",
                                             ins=[], outs=[])
                        nop.engine = ins.engine
                        nop.sync_info = mybir.SyncInfo(on_wait=chunk, on_update=[])
                        new_list.append(nop)
                    si.on_wait = keep
                    changed = True
                new_list.append(ins)
            if changed:
                b.instructions = new_list


def _build(tb, sbatch=16, xslab_tiles=64):
    import concourse.bass as bass
    import concourse.mybir as mybir
    import concourse.tile as tile
    F32 = mybir.dt.float32
    BF16 = mybir.dt.bfloat16
    I32 = mybir.dt.int32
    AF = mybir.ActivationFunctionType
    NT = NB * tb
    nc = bass.Bass()

    # Packed input blobs: the per-call dispatch overhead scales with the
    # number of parameters (~1.2 ms each), so all 18 logical inputs live in
    # TWO blobs (128-partition data in cb16, with f32/int32 sections stored
    # as raw bytes and bitcast device-side; the 37-partition x^T remainder
    # in cb37) plus the output.
    WF = HID + HID + 2 + NT1 + N_CORES * NB + NB   # b1|b2|bc|dinv_nt|dinv_l2|dinv_blk
    W16 = NPAD + HID + HID + 2 + P + P + NT + 2 * WF + 4 * NT
    W37 = NPAD + HID                               # xt_b|w1b
    cb16 = nc.declare_dram_parameter("cb16", [P, W16], BF16, isOutput=False)
    cb37 = nc.declare_dram_parameter("cb37", [IN_F - P, W37], BF16, isOutput=False)
    out = nc.declare_dram_parameter("out", [SHARD, 2], F32, isOutput=True)

    def _sl(param, off, w):
        return param[:, off:off + w]

    o16_w1a = NPAD
    o16_w2 = o16_w1a + HID
    o16_wc = o16_w2 + HID
    o16_iota = o16_wc + 2
    o16_id = o16_iota + P
    o16_dst = o16_id + P
    o_f32 = o16_dst + NT           # f32 section, bf16 units (2 per f32)
    o_i32 = o_f32 + 2 * WF         # i32 section, bf16 units (2 per i32)
    assert W16 % 2 == 0 and o_f32 % 2 == 0 and o_i32 % 2 == 0

    def _f32(off, w):
        return cb16[:, o_f32 + 2 * off:o_f32 + 2 * (off + w)].bitcast(F32)

    def _i32(off, w):
        return cb16[:, o_i32 + 2 * off:o_i32 + 2 * (off + w)].bitcast(I32)

    of_b1, of_b2, of_bc = 0, HID, 2 * HID
    of_dnt = of_bc + 2
    of_dl2 = of_dnt + NT1
    of_dbk = of_dl2 + N_CORES * NB
    xt_a = cb16      # columns [0, NPAD)
    xt_b = cb37      # columns [0, NPAD)

    with tile.TileContext(nc) as tc:
        with tc.tile_pool(name="const", bufs=1) as constp, \
             tc.tile_pool(name="dram", bufs=1, space="DRAM") as dramp:

            def load_const(name, ap, shape, dt):
                t = constp.tile(shape, dt, name=name)
                nc.sync.dma_start(out=t[:], in_=ap)
                return t

            w1a_s = load_const("w1a_s", _sl(cb16, o16_w1a, HID), [P, HID], BF16)
            w1b_s = load_const("w1b_s", cb37[:, NPAD:NPAD + HID],
                               [IN_F - P, HID], BF16)
            w2_s = load_const("w2_s", _sl(cb16, o16_w2, HID), [HID, HID], BF16)
            wc_s = load_const("wc_s", _sl(cb16, o16_wc, 2), [HID, 2], BF16)
            b1_s = load_const("b1_s", _f32(of_b1, HID), [P, HID], F32)
            b2_s = load_const("b2_s", _f32(of_b2, HID), [P, HID], F32)
            bc_s = load_const("bc_s", _f32(of_bc, 2), [P, 2], F32)
            iota_s = load_const("iota_s", _sl(cb16, o16_iota, P), [P, P], BF16)
            id_s = load_const("id_s", _sl(cb16, o16_id, P), [P, P], BF16)
            dinv_nt_s = load_const("dinv_nt_s", _f32(of_dnt, NT1),
                                   [P, NT1], F32)
            dinv_l2_s = load_const("dinv_l2_s", _f32(of_dl2, N_CORES * NB),
                                   [P, N_CORES * NB], F32)
            dinv_blk_s = load_const("dinv_blk_s", _f32(of_dbk, NB),
                                    [P, NB], F32)
            dstloc_s = load_const("dstloc_s", _sl(cb16, o16_dst, NT),
                                  [P, NT], BF16)
            idx1_s = load_const("idx1_s", _i32(0, NT), [P, NT], I32)
            idx2_s = load_const("idx2_s", _i32(NT, NT), [P, NT], I32)

            g1_t = dramp.tile([NPAD, HID], BF16, name="g1_t")
            g2_t = dramp.tile([NPAD2, HID], BF16, name="g2_t")
            a1T_t = dramp.tile([P, SHARD_PAD], BF16, name="a1T_t")
            ag_t = dramp.tile([N_CORES * P, SHARD_PAD], BF16,
                              addr_space="Shared", name="ag_t")

            # ---------------- L1 dense ----------------
            with tc.tile_pool(name="xslab", bufs=2) as xsp, \
                 tc.tile_pool(name="gstage", bufs=2) as gsp, \
                 tc.tile_pool(name="psD", bufs=4, space="PSUM") as psD:
                t0 = 0
                while t0 < NT1:
                    tn = min(xslab_tiles, NT1 - t0)
                    xa = xsp.tile([P, xslab_tiles * P], BF16, tag="xa", name="xa")
                    nc.sync.dma_start(out=xa[:, :tn * P],
                                      in_=xt_a[:, t0 * P:(t0 + tn) * P])
                    xb = xsp.tile([IN_F - P, xslab_tiles * P], BF16, tag="xb",
                                  name="xb")
                    nc.sync.dma_start(out=xb[:, :tn * P],
                                      in_=xt_b[:, t0 * P:(t0 + tn) * P])
                    gst = gsp.tile([P, xslab_tiles * HID], BF16, tag="gst",
                                   name="gst")
                    for i in range(tn):
                        ps = psD.tile([P, HID], F32, tag="psD", name="psd")
                        nc.tensor.matmul(ps[:], lhsT=xa[:, i * P:(i + 1) * P],
                                         rhs=w1a_s[:], start=True, stop=False)
                        nc.tensor.matmul(ps[:], lhsT=xb[:, i * P:(i + 1) * P],
                                         rhs=w1b_s[:], start=False, stop=True)
                        nc.scalar.activation(gst[:, i * HID:(i + 1) * HID], ps[:],
                                             AF.Copy,
                                             scale=dinv_nt_s[:, t0 + i:t0 + i + 1])
                    nc.sync.dma_start(
                        out=g1_t[t0 * P:(t0 + tn) * P, :].rearrange(
                            "(t p) f -> p t f", p=P),
                        in_=gst[:, :tn * HID].rearrange("p (t f) -> p t f", f=HID))
                    t0 += tn

            def aggregation(g_tab, idx_s, bias_s, out_blk_cb, phase):
                with tc.tile_pool(name=f"gt{phase}", bufs=16) as gtp, \
                     tc.tile_pool(name=f"sp{phase}", bufs=3) as spp, \
                     tc.tile_pool(name=f"ps{phase}", bufs=4, space="PSUM") as psp, \
                     tc.tile_pool(name=f"ep{phase}", bufs=3) as epp:
                    S = None
                    for b in range(NB):
                        ps = psp.tile([P, HID], F32, tag="ps", name="ps")
                        for t in range(tb):
                            gi = b * tb + t
                            if gi % sbatch == 0:
                                nw = min(sbatch, NT - gi)
                                S = spp.tile([P, sbatch * P], BF16, tag="S",
                                             name="S")
                                iota_b = bass.AP(
                                    iota_s[:].tensor, iota_s[:].offset,
                                    [iota_s[:].ap[0], [0, nw], iota_s[:].ap[1]])
                                dst_b = dstloc_s[:, gi:gi + nw].to_broadcast(
                                    [P, nw, P])
                                nc.vector.tensor_tensor(
                                    out=S[:, :nw * P].rearrange(
                                        "p (t f) -> p t f", f=P),
                                    in0=iota_b, in1=dst_b,
                                    op=mybir.AluOpType.is_equal)
                            G = gtp.tile([P, HID], BF16, tag="G", name="G")
                            nc.gpsimd.indirect_dma_start(
                                out=G[:], out_offset=None, in_=g_tab[:],
                                in_offset=bass.IndirectOffsetOnAxis(
                                    ap=idx_s[:, gi:gi + 1], axis=0))
                            nc.tensor.matmul(
                                ps[:],
                                lhsT=S[:, (gi % sbatch) * P:(gi % sbatch + 1) * P],
                                rhs=G[:], start=(t == 0), stop=(t == tb - 1))
                        t1 = epp.tile([P, HID], F32, tag="t1", name="t1")
                        nc.scalar.activation(t1[:], ps[:], AF.Copy,
                                             scale=dinv_blk_s[:, b:b + 1])
                        t2 = epp.tile([P, HID], F32, tag="t2", name="t2")
                        nc.vector.tensor_tensor(out=t2[:], in0=t1[:], in1=bias_s[:],
                                                op=mybir.AluOpType.add)
                        a_sb = epp.tile([P, HID], BF16, tag="a_sb", name="a_sb")
                        nc.vector.tensor_scalar_max(out=a_sb[:], in0=t2[:],
                                                    scalar1=0.0)
                        out_blk_cb(b, a_sb)

            # ---------------- L1 agg -> a1T -> AllGather ----------------
            with tc.tile_pool(name="a1Ts", bufs=1) as a1sp, \
                 tc.tile_pool(name="psT", bufs=2, space="PSUM") as psT:
                a1T_stage = a1sp.tile([P, SHARD_PAD], BF16, name="a1T_stage")

                def l1_out(b, a_sb):
                    pT = psT.tile([P, P], BF16, tag="pT", name="pT")
                    nc.tensor.transpose(pT[:], a_sb[:], id_s[:])
                    nc.vector.tensor_copy(out=a1T_stage[:, b * P:(b + 1) * P],
                                          in_=pT[:])

                aggregation(g1_t, idx1_s, b1_s, l1_out, "A")
                nc.sync.dma_start(out=a1T_t[:], in_=a1T_stage[:])

            nc.gpsimd.collective_compute(
                "AllGather", mybir.AluOpType.bypass,
                replica_groups=[list(range(N_CORES))],
                ins=[a1T_t[:]], outs=[ag_t[:]])

            # ---------------- L2 dense ----------------
            with tc.tile_pool(name="l2slab", bufs=2) as l2sp, \
                 tc.tile_pool(name="gstage2", bufs=2) as gsp2, \
                 tc.tile_pool(name="psD2", bufs=4, space="PSUM") as psD2:
                for cb in range(N_CORES):
                    t0 = 0
                    while t0 < NB:
                        tn = min(32, NB - t0)
                        lh = l2sp.tile([P, 32 * P], BF16, tag="lh", name="lh")
                        nc.sync.dma_start(out=lh[:, :tn * P],
                                          in_=ag_t[cb * P:(cb + 1) * P,
                                                   t0 * P:(t0 + tn) * P])
                        gst = gsp2.tile([P, 32 * HID], BF16, tag="gst2",
                                        name="gst2")
                        for i in range(tn):
                            ps = psD2.tile([P, HID], F32, tag="psD2", name="psd2")
                            nc.tensor.matmul(ps[:], lhsT=lh[:, i * P:(i + 1) * P],
                                             rhs=w2_s[:], start=True, stop=True)
                            nc.scalar.activation(
                                gst[:, i * HID:(i + 1) * HID], ps[:], AF.Copy,
                                scale=dinv_l2_s[:, cb * NB + t0 + i:
                                                cb * NB + t0 + i + 1])
                        r0 = cb * SHARD_PAD + t0 * P
                        nc.sync.dma_start(
                            out=g2_t[r0:r0 + tn * P, :].rearrange(
                                "(t p) f -> p t f", p=P),
                            in_=gst[:, :tn * HID].rearrange("p (t f) -> p t f",
                                                            f=HID))
                        t0 += tn

            # ---------------- L2 agg + classifier ----------------
            with tc.tile_pool(name="psT2", bufs=2, space="PSUM") as psT2, \
                 tc.tile_pool(name="clsb", bufs=3) as clsp, \
                 tc.tile_pool(name="psC", bufs=2, space="PSUM") as psC:

                def l2_out(b, a_sb):
                    pT = psT2.tile([P, P], BF16, tag="pT2", name="pT2")
                    nc.tensor.transpose(pT[:], a_sb[:], id_s[:])
                    a2T = clsp.tile([P, P], BF16, tag="a2T", name="a2T")
                    nc.vector.tensor_copy(out=a2T[:], in_=pT[:])
                    pc = psC.tile([P, 2], F32, tag="pC", name="pC")
                    nc.tensor.matmul(pc[:], lhsT=a2T[:], rhs=wc_s[:],
                                     start=True, stop=True)
                    ob = clsp.tile([P, 2], F32, tag="ob", name="ob")
                    nc.vector.tensor_tensor(out=ob[:], in0=pc[:], in1=bc_s[:],
                                            op=mybir.AluOpType.add)
                    nrows = min(P, SHARD - b * P)
                    nc.sync.dma_start(out=out[b * P:b * P + nrows, :],
                                      in_=ob[:nrows, :])

                aggregation(g2_t, idx2_s, b2_s, l2_out, "B")

    mybir.codegen_inst_isa_subclasses(nc)
    _split_excess_waits(nc)
    return nc


def _prepare(x, edge_index, W1, b1, W2, b2, Wc, bc):
    import ml_dtypes
    bf = ml_dtypes.bfloat16
    x = np.asarray(x, np.float32)
    src = np.asarray(edge_index[0], dtype=np.int64)
    dst = np.asarray(edge_index[1], dtype=np.int64)
    deg = np.bincount(dst, minlength=N).astype(np.float32) + 1.0
    dinv = 1.0 / np.sqrt(deg)
    allsrc = np.concatenate([src, np.arange(N, dtype=np.int64)])
    alldst = np.concatenate([dst, np.arange(N, dtype=np.int64)])

    # Balance per-block edge counts: permute each core's local node
    # positions (snake assignment by degree) so every 128-node dst block has
    # a near-equal edge count -> smaller uniform tiles-per-block (tb).
    import heapq
    caps = np.full(NB, P, np.int64)
    caps[-1] = SHARD - (NB - 1) * P
    perms = []
    for cc in range(N_CORES):
        lo = cc * SHARD
        order = np.argsort(-deg[lo:lo + SHARD], kind="stable")
        heap = [(0.0, int(b)) for b in range(NB)]
        heapq.heapify(heap)
        fill = np.zeros(NB, np.int64)
        perm = np.empty(SHARD, np.int64)
        degs = deg[lo:lo + SHARD]
        for ol in order:
            while True:
                w, b = heapq.heappop(heap)
                if fill[b] < caps[b]:
                    break
            perm[ol] = b * P + fill[b]
            fill[b] += 1
            if fill[b] < caps[b]:
                heapq.heappush(heap, (w + float(degs[ol]), b))
        perms.append(perm)

    per_core = []
    tb = 0
    for cc in range(N_CORES):
        lo, hi = cc * SHARD, (cc + 1) * SHARD
        m = (alldst >= lo) & (alldst < hi)
        s = allsrc[m]
        d = perms[cc][alldst[m] - lo]       # permuted local positions
        blk = d // P
        order = np.argsort(blk, kind="stable")
        s, d, blk = s[order], d[order], blk[order]
        counts = np.bincount(blk, minlength=NB)
        tb = max(tb, int(np.ceil(counts.max() / P)))
        per_core.append((s, d, counts))
    NT = NB * tb

    dinv_pad = np.ones(NPAD, np.float32)
    dinv_pad[:N] = dinv
    xt = np.zeros((IN_F, NPAD), np.float32)
    xt[:, :N] = x.T
    w1f = np.asarray(W1, np.float32)
    cb16_common = np.concatenate([
        xt[:P].astype(bf),
        w1f[:P].astype(bf),
        np.asarray(W2, np.float32).astype(bf),
        np.asarray(Wc, np.float32).astype(bf),
        np.broadcast_to(np.arange(P, dtype=np.float32), (P, P)).astype(bf),
        np.eye(P, dtype=np.float32).astype(bf),
    ], axis=1)
    cb37 = np.concatenate([xt[P:].astype(bf), w1f[P:].astype(bf)], axis=1)
    d2 = np.ones((N_CORES, SHARD_PAD), np.float32)
    for cc in range(N_CORES):
        d2[cc, perms[cc]] = dinv[cc * SHARD:(cc + 1) * SHARD]
    cf32_common = np.concatenate([
        np.broadcast_to(np.asarray(b1, np.float32), (P, HID)),
        np.broadcast_to(np.asarray(b2, np.float32), (P, HID)),
        np.broadcast_to(np.asarray(bc, np.float32), (P, 2)),
        dinv_pad.reshape(NT1, P).T,
        d2.reshape(N_CORES * NB, P).T,
    ], axis=1).astype(np.float32)

    in_maps = []
    for cc in range(N_CORES):
        s, d, counts = per_core[cc]
        idx_u = np.zeros((NT, P), np.int64)
        dst_u = np.full((NT, P), -1.0, np.float32)
        offs = np.concatenate([[0], np.cumsum(counts)])
        for b in range(NB):
            es = s[offs[b]:offs[b + 1]]
            ed = d[offs[b]:offs[b + 1]] % P
            nloc = len(es)
            buf_s = np.zeros(tb * P, np.int64)
            buf_d = np.full(tb * P, -1.0, np.float32)
            buf_s[:nloc] = es
            buf_d[:nloc] = ed
            idx_u[b * tb:(b + 1) * tb] = buf_s.reshape(tb, P)
            dst_u[b * tb:(b + 1) * tb] = buf_d.reshape(tb, P)
        idx_u = idx_u.T.copy()
        dst_u = dst_u.T.copy()
        src_core = idx_u // SHARD
        src_newpos = np.empty_like(idx_u)
        for c2 in range(N_CORES):
            sel = src_core == c2
            src_newpos[sel] = perms[c2][idx_u[sel] % SHARD]
        idx2_u = src_core * SHARD_PAD + src_newpos
        lo = cc * SHARD
        dv = np.ones(SHARD_PAD, np.float32)
        dv[perms[cc]] = dinv[lo:lo + SHARD]
        f32_part = np.ascontiguousarray(np.concatenate(
            [cf32_common, dv.reshape(NB, P).T], axis=1).astype(np.float32))
        i32_part = np.ascontiguousarray(
            np.concatenate([idx_u, idx2_u], axis=1).astype(np.int32))
        m = {
            "cb16": np.concatenate([cb16_common, dst_u.astype(bf),
                                    f32_part.view(bf), i32_part.view(bf)],
                                   axis=1),
            "cb37": cb37,
        }
        in_maps.append(m)
    return tb, in_maps, perms


class _Runner:
    """Compile the Bass SPMD program once and execute it on cores 0-7 via
    the PJRT path (modeled on concourse.bass2jax.run_bass_via_pjrt)."""

    def __init__(self, nc, n_cores=8):
        import jax
        import concourse.mybir as mybir
        from jax.sharding import Mesh, PartitionSpec
        from jax.experimental.shard_map import shard_map
        from concourse.bass2jax import (_bass_exec_p, partition_id_tensor,
                                        install_neuronx_cc_hook)
        install_neuronx_cc_hook()
        self.jax = jax
        self.n_cores = n_cores
        in_names, out_names, out_avals = [], [], []
        partition_name = (nc.partition_id_tensor.name
                          if nc.partition_id_tensor else None)
        for alloc in nc.m.functions[0].allocations:
            if not isinstance(alloc, mybir.MemoryLocationSet):
                continue
            name = alloc.memorylocations[0].name
            if alloc.kind == "ExternalInput":
                if name != partition_name:
                    in_names.append(name)
            elif alloc.kind == "ExternalOutput":
                out_names.append(name)
                out_avals.append(jax.core.ShapedArray(
                    tuple(alloc.tensor_shape), mybir.dt.np(alloc.dtype)))
        self.in_names, self.out_names, self.out_avals = \
            in_names, out_names, out_avals
        n_params = len(in_names)
        all_in = list(in_names) + list(out_names)
        if partition_name is not None:
            all_in.append(partition_name)

        def _body(*args):
            operands = list(args)
            if partition_name is not None:
                operands.append(partition_id_tensor())
            outs = _bass_exec_p.bind(
                *operands, out_avals=tuple(out_avals), in_names=tuple(all_in),
                out_names=tuple(out_names), lowering_input_output_aliases=(),
                sim_require_finite=True, sim_require_nnan=True, nc=nc)
            return tuple(outs)

        devices = jax.devices()[:n_cores]
        self.mesh = Mesh(np.asarray(devices), ("core",))
        n_outs = len(out_avals)
        in_specs = (PartitionSpec("core"),) * (n_params + n_outs)
        out_specs = (PartitionSpec("core"),) * n_outs
        self.fn = jax.jit(
            shard_map(_body, mesh=self.mesh, in_specs=in_specs,
                      out_specs=out_specs, check_rep=False),
            keep_unused=True)

    def prep_inputs(self, in_maps):
        import jax
        from jax.sharding import NamedSharding, PartitionSpec
        concat = [np.concatenate([np.asarray(m[name]) for m in in_maps], axis=0)
                  for name in self.in_names]
        zeros = [np.zeros((self.n_cores * a.shape[0], *a.shape[1:]), a.dtype)
                 for a in self.out_avals]
        sharding = NamedSharding(self.mesh, PartitionSpec("core"))
        return [jax.device_put(a, sharding) for a in concat + zeros]

    def run(self, dev_args):
        outs = self.fn(*dev_args)
        self.jax.block_until_ready(outs)
        return outs

    def results(self, outs):
        res = []
        for c in range(self.n_cores):
            d = {}
            for i, name in enumerate(self.out_names):
                d[name] = np.asarray(outs[i]).reshape(
                    self.n_cores, *self.out_avals[i].shape)[c]
            res.append(d)
        return res


_CACHED = {}


def kernel(x, edge_index, W1, b1, W2, b2, Wc, bc):
    tb, in_maps, perms = _prepare(x, edge_index, W1, b1, W2, b2, Wc, bc)
    key = tb
    if key not in _CACHED:
        nc = _build(tb)
        _CACHED[key] = _Runner(nc)
    r = _CACHED[key]
    dev = r.prep_inputs(in_maps)
    outs = r.run(dev)
    res = r.results(outs)
    full = np.concatenate([res[c]["out"][perms[c]] for c in range(N_CORES)],
                          axis=0)
    return full.astype(np.float32)



# revision 11
# speedup vs baseline: 1.3295x; 1.3295x over previous
"""Trainium2 Bass kernel for a 2-layer GCN (AblationGCN) on 8 NeuronCores.

Contract: kernel(**inputs) takes the FULL unsharded inputs of the reference
(x [100000,165] f32, edge_index [2,1600000] int, W1,b1,W2,b2,Wc,bc) and
returns the FULL output [100000, 2] f32.

Strategy (SPMD, one Bass program on cores 0-7):
  - dst-range sharding: core c owns output nodes [c*12500, (c+1)*12500).
  - L1 dense (replicated): h1 = x @ W1 for all nodes via TensorE with
    host-transposed x; g1 = dinv*h1 stored as a bf16 node-major DRAM table.
  - L1 aggregation: edges (+self loops) grouped by dst block (128 nodes),
    each block padded to a uniform TB tiles of 128 edges. Per tile:
    indirect-DMA gather of g1[src] rows, a one-hot selection matrix S
    (iota == dstlocal on VectorE), and a TensorE matmul S^T @ G
    accumulating the block in PSUM. Epilogue: *dinv, +bias, relu.
  - a1 blocks are PE-transposed and exchanged with one AllGather.
  - L2 dense from the allgathered feature-major a1T; L2 aggregation same
    as L1; classifier a2T.T @ Wc + bc per block.
Host preprocessing computes degrees/dinv and the per-core uniform tile
metadata (indices, local dst one-hot keys).
"""
import numpy as np

P = 128
N = 100000
N_CORES = 8
IN_F = 165
HID = 128
SHARD = N // N_CORES            # 12500
NB = (SHARD + P - 1) // P       # 98
NPAD = ((N + P - 1) // P) * P   # 100096
SHARD_PAD = NB * P              # 12544
NPAD2 = N_CORES * SHARD_PAD     # 100352
NT1 = NPAD // P                 # 782


def _split_excess_waits(nc, max_waits=1):
    """This walrus build only accepts one sync-wait command per instruction;
    hoist extras onto NoOps placed just before the carrying instruction."""
    import concourse.mybir as mybir
    for f in nc.m.functions:
        for b in f.blocks:
            insts = b.instructions
            new_list = []
            changed = False
            for ins in insts:
                si = ins.sync_info
                if si is not None and len(si.on_wait) > max_waits:
                    waits = list(si.on_wait)
                    keep = waits[:max_waits]
                    extra = waits[max_waits:]
                    for ci in range(0, len(extra), max_waits):
                        chunk = extra[ci:ci + max_waits]
                        nop = mybir.InstNoOp(name=f"{ins.name}_wsplit{ci}",
                                             ins=[], outs=[])
                        nop.engine = ins.engine
                        nop.sync_info = mybir.SyncInfo(on_wait=chunk, on_update=[])
                        new_list.append(nop)
                    si.on_wait = keep
                    changed = True
                new_list.append(ins)
            if changed:
                b.instructions = new_list


def _build(tb, sbatch=16, xslab_tiles=64):
    import concourse.bass as bass
    import concourse.mybir as mybir
    import concourse.tile as tile
    F32 = mybir.dt.float32
    BF16 = mybir.dt.bfloat16
    I32 = mybir.dt.int32
    AF = mybir.ActivationFunctionType
    NT = NB * tb
    nc = bass.Bass()

    # Packed input blobs: the per-call dispatch overhead scales with the
    # number of parameters (~1.2 ms each), so all 18 logical inputs live in
    # TWO blobs (128-partition data in cb16, with f32/int32 sections stored
    # as raw bytes and bitcast device-side; the 37-partition x^T remainder
    # in cb37) plus the output.
    WF = HID + HID + 2 + NT1 + N_CORES * NB + NB   # b1|b2|bc|dinv_nt|dinv_l2|dinv_blk
    W16 = NPAD + HID + HID + 2 + P + P + NT + 2 * WF + 4 * NT
    W37 = NPAD + HID                               # xt_b|w1b
    cb16 = nc.declare_dram_parameter("cb16", [P, W16], BF16, isOutput=False)
    cb37 = nc.declare_dram_parameter("cb37", [IN_F - P, W37], BF16, isOutput=False)
    out = nc.declare_dram_parameter("out", [SHARD, 2], F32, isOutput=True)

    def _sl(param, off, w):
        return param[:, off:off + w]

    o16_w1a = NPAD
    o16_w2 = o16_w1a + HID
    o16_wc = o16_w2 + HID
    o16_iota = o16_wc + 2
    o16_id = o16_iota + P
    o16_dst = o16_id + P
    o_f32 = o16_dst + NT           # f32 section, bf16 units (2 per f32)
    o_i32 = o_f32 + 2 * WF         # i32 section, bf16 units (2 per i32)
    assert W16 % 2 == 0 and o_f32 % 2 == 0 and o_i32 % 2 == 0

    def _f32(off, w):
        return cb16[:, o_f32 + 2 * off:o_f32 + 2 * (off + w)].bitcast(F32)

    def _i32(off, w):
        return cb16[:, o_i32 + 2 * off:o_i32 + 2 * (off + w)].bitcast(I32)

    of_b1, of_b2, of_bc = 0, HID, 2 * HID
    of_dnt = of_bc + 2
    of_dl2 = of_dnt + NT1
    of_dbk = of_dl2 + N_CORES * NB
    xt_a = cb16      # columns [0, NPAD)
    xt_b = cb37      # columns [0, NPAD)

    with tile.TileContext(nc) as tc:
        with tc.tile_pool(name="const", bufs=1) as constp, \
             tc.tile_pool(name="dram", bufs=1, space="DRAM") as dramp:

            def load_const(name, ap, shape, dt):
                t = constp.tile(shape, dt, name=name)
                nc.sync.dma_start(out=t[:], in_=ap)
                return t

            w1a_s = load_const("w1a_s", _sl(cb16, o16_w1a, HID), [P, HID], BF16)
            w1b_s = load_const("w1b_s", cb37[:, NPAD:NPAD + HID],
                               [IN_F - P, HID], BF16)
            w2_s = load_const("w2_s", _sl(cb16, o16_w2, HID), [HID, HID], BF16)
            wc_s = load_const("wc_s", _sl(cb16, o16_wc, 2), [HID, 2], BF16)
            b1_s = load_const("b1_s", _f32(of_b1, HID), [P, HID], F32)
            b2_s = load_const("b2_s", _f32(of_b2, HID), [P, HID], F32)
            bc_s = load_const("bc_s", _f32(of_bc, 2), [P, 2], F32)
            iota_s = load_const("iota_s", _sl(cb16, o16_iota, P), [P, P], BF16)
            id_s = load_const("id_s", _sl(cb16, o16_id, P), [P, P], BF16)
            dinv_nt_s = load_const("dinv_nt_s", _f32(of_dnt, NT1),
                                   [P, NT1], F32)
            dinv_l2_s = load_const("dinv_l2_s", _f32(of_dl2, N_CORES * NB),
                                   [P, N_CORES * NB], F32)
            dinv_blk_s = load_const("dinv_blk_s", _f32(of_dbk, NB),
                                    [P, NB], F32)
            dstloc_s = load_const("dstloc_s", _sl(cb16, o16_dst, NT),
                                  [P, NT], BF16)
            idx1_s = load_const("idx1_s", _i32(0, NT), [P, NT], I32)
            idx2_s = load_const("idx2_s", _i32(NT, NT), [P, NT], I32)

            g1_t = dramp.tile([NPAD, HID], BF16, name="g1_t")
            g2_t = dramp.tile([NPAD2, HID], BF16, name="g2_t")
            a1T_t = dramp.tile([P, SHARD_PAD], BF16, name="a1T_t")
            ag_t = dramp.tile([N_CORES * P, SHARD_PAD], BF16,
                              addr_space="Shared", name="ag_t")

            # ---------------- L1 dense ----------------
            with tc.tile_pool(name="xslab", bufs=2) as xsp, \
                 tc.tile_pool(name="gstage", bufs=2) as gsp, \
                 tc.tile_pool(name="psD", bufs=4, space="PSUM") as psD:
                t0 = 0
                while t0 < NT1:
                    tn = min(xslab_tiles, NT1 - t0)
                    xa = xsp.tile([P, xslab_tiles * P], BF16, tag="xa", name="xa")
                    nc.sync.dma_start(out=xa[:, :tn * P],
                                      in_=xt_a[:, t0 * P:(t0 + tn) * P])
                    xb = xsp.tile([IN_F - P, xslab_tiles * P], BF16, tag="xb",
                                  name="xb")
                    nc.sync.dma_start(out=xb[:, :tn * P],
                                      in_=xt_b[:, t0 * P:(t0 + tn) * P])
                    gst = gsp.tile([P, xslab_tiles * HID], BF16, tag="gst",
                                   name="gst")
                    for i in range(tn):
                        ps = psD.tile([P, HID], F32, tag="psD", name="psd")
                        nc.tensor.matmul(ps[:], lhsT=xa[:, i * P:(i + 1) * P],
                                         rhs=w1a_s[:], start=True, stop=False)
                        nc.tensor.matmul(ps[:], lhsT=xb[:, i * P:(i + 1) * P],
                                         rhs=w1b_s[:], start=False, stop=True)
                        nc.scalar.activation(gst[:, i * HID:(i + 1) * HID], ps[:],
                                             AF.Copy,
                                             scale=dinv_nt_s[:, t0 + i:t0 + i + 1])
                    nc.sync.dma_start(
                        out=g1_t[t0 * P:(t0 + tn) * P, :].rearrange(
                            "(t p) f -> p t f", p=P),
                        in_=gst[:, :tn * HID].rearrange("p (t f) -> p t f", f=HID))
                    t0 += tn

            def aggregation(g_tab, idx_s, bias_s, out_blk_cb, phase):
                with tc.tile_pool(name=f"gt{phase}", bufs=16) as gtp, \
                     tc.tile_pool(name=f"sp{phase}", bufs=3) as spp, \
                     tc.tile_pool(name=f"ps{phase}", bufs=4, space="PSUM") as psp, \
                     tc.tile_pool(name=f"ep{phase}", bufs=3) as epp:
                    S = None
                    for b in range(NB):
                        ps = psp.tile([P, HID], F32, tag="ps", name="ps")
                        for t in range(tb):
                            gi = b * tb + t
                            if gi % sbatch == 0:
                                nw = min(sbatch, NT - gi)
                                S = spp.tile([P, sbatch * P], BF16, tag="S",
                                             name="S")
                                iota_b = bass.AP(
                                    iota_s[:].tensor, iota_s[:].offset,
                                    [iota_s[:].ap[0], [0, nw], iota_s[:].ap[1]])
                                dst_b = dstloc_s[:, gi:gi + nw].to_broadcast(
                                    [P, nw, P])
                                nc.vector.tensor_tensor(
                                    out=S[:, :nw * P].rearrange(
                                        "p (t f) -> p t f", f=P),
                                    in0=iota_b, in1=dst_b,
                                    op=mybir.AluOpType.is_equal)
                            G = gtp.tile([P, HID], BF16, tag="G", name="G")
                            nc.gpsimd.indirect_dma_start(
                                out=G[:], out_offset=None, in_=g_tab[:],
                                in_offset=bass.IndirectOffsetOnAxis(
                                    ap=idx_s[:, gi:gi + 1], axis=0))
                            nc.tensor.matmul(
                                ps[:],
                                lhsT=S[:, (gi % sbatch) * P:(gi % sbatch + 1) * P],
                                rhs=G[:], start=(t == 0), stop=(t == tb - 1))
                        t1 = epp.tile([P, HID], F32, tag="t1", name="t1")
                        nc.scalar.activation(t1[:], ps[:], AF.Copy,
                                             scale=dinv_blk_s[:, b:b + 1])
                        t2 = epp.tile([P, HID], F32, tag="t2", name="t2")
                        nc.vector.tensor_tensor(out=t2[:], in0=t1[:], in1=bias_s[:],
                                                op=mybir.AluOpType.add)
                        a_sb = epp.tile([P, HID], BF16, tag="a_sb", name="a_sb")
                        nc.vector.tensor_scalar_max(out=a_sb[:], in0=t2[:],
                                                    scalar1=0.0)
                        out_blk_cb(b, a_sb)

            # ---------------- L1 agg -> a1T -> AllGather ----------------
            with tc.tile_pool(name="a1Ts", bufs=1) as a1sp, \
                 tc.tile_pool(name="psT", bufs=2, space="PSUM") as psT:
                a1T_stage = a1sp.tile([P, SHARD_PAD], BF16, name="a1T_stage")

                def l1_out(b, a_sb):
                    pT = psT.tile([P, P], BF16, tag="pT", name="pT")
                    nc.tensor.transpose(pT[:], a_sb[:], id_s[:])
                    nc.vector.tensor_copy(out=a1T_stage[:, b * P:(b + 1) * P],
                                          in_=pT[:])

                aggregation(g1_t, idx1_s, b1_s, l1_out, "A")
                nc.sync.dma_start(out=a1T_t[:], in_=a1T_stage[:])

            nc.gpsimd.collective_compute(
                "AllGather", mybir.AluOpType.bypass,
                replica_groups=[list(range(N_CORES))],
                ins=[a1T_t[:]], outs=[ag_t[:]])

            # ---------------- L2 dense ----------------
            with tc.tile_pool(name="l2slab", bufs=2) as l2sp, \
                 tc.tile_pool(name="gstage2", bufs=2) as gsp2, \
                 tc.tile_pool(name="psD2", bufs=4, space="PSUM") as psD2:
                for cb in range(N_CORES):
                    t0 = 0
                    while t0 < NB:
                        tn = min(32, NB - t0)
                        lh = l2sp.tile([P, 32 * P], BF16, tag="lh", name="lh")
                        nc.sync.dma_start(out=lh[:, :tn * P],
                                          in_=ag_t[cb * P:(cb + 1) * P,
                                                   t0 * P:(t0 + tn) * P])
                        gst = gsp2.tile([P, 32 * HID], BF16, tag="gst2",
                                        name="gst2")
                        for i in range(tn):
                            ps = psD2.tile([P, HID], F32, tag="psD2", name="psd2")
                            nc.tensor.matmul(ps[:], lhsT=lh[:, i * P:(i + 1) * P],
                                             rhs=w2_s[:], start=True, stop=True)
                            nc.scalar.activation(
                                gst[:, i * HID:(i + 1) * HID], ps[:], AF.Copy,
                                scale=dinv_l2_s[:, cb * NB + t0 + i:
                                                cb * NB + t0 + i + 1])
                        r0 = cb * SHARD_PAD + t0 * P
                        nc.sync.dma_start(
                            out=g2_t[r0:r0 + tn * P, :].rearrange(
                                "(t p) f -> p t f", p=P),
                            in_=gst[:, :tn * HID].rearrange("p (t f) -> p t f",
                                                            f=HID))
                        t0 += tn

            # ---------------- L2 agg + classifier ----------------
            with tc.tile_pool(name="psT2", bufs=2, space="PSUM") as psT2, \
                 tc.tile_pool(name="clsb", bufs=3) as clsp, \
                 tc.tile_pool(name="psC", bufs=2, space="PSUM") as psC:

                def l2_out(b, a_sb):
                    pT = psT2.tile([P, P], BF16, tag="pT2", name="pT2")
                    nc.tensor.transpose(pT[:], a_sb[:], id_s[:])
                    a2T = clsp.tile([P, P], BF16, tag="a2T", name="a2T")
                    nc.vector.tensor_copy(out=a2T[:], in_=pT[:])
                    pc = psC.tile([P, 2], F32, tag="pC", name="pC")
                    nc.tensor.matmul(pc[:], lhsT=a2T[:], rhs=wc_s[:],
                                     start=True, stop=True)
                    ob = clsp.tile([P, 2], F32, tag="ob", name="ob")
                    nc.vector.tensor_tensor(out=ob[:], in0=pc[:], in1=bc_s[:],
                                            op=mybir.AluOpType.add)
                    nrows = min(P, SHARD - b * P)
                    nc.sync.dma_start(out=out[b * P:b * P + nrows, :],
                                      in_=ob[:nrows, :])

                aggregation(g2_t, idx2_s, b2_s, l2_out, "B")

    mybir.codegen_inst_isa_subclasses(nc)
    _split_excess_waits(nc)
    return nc


def _prepare(x, edge_index, W1, b1, W2, b2, Wc, bc):
    import ml_dtypes
    bf = ml_dtypes.bfloat16
    x = np.asarray(x, np.float32)
    src = np.asarray(edge_index[0], dtype=np.int64)
    dst = np.asarray(edge_index[1], dtype=np.int64)
    deg = np.bincount(dst, minlength=N).astype(np.float32) + 1.0
    dinv = 1.0 / np.sqrt(deg)
    allsrc = np.concatenate([src, np.arange(N, dtype=np.int64)])
    alldst = np.concatenate([dst, np.arange(N, dtype=np.int64)])

    # Balance per-block edge counts: permute each core's local node
    # positions (snake assignment by degree) so every 128-node dst block has
    # a near-equal edge count -> smaller uniform tiles-per-block (tb).
    import heapq
    caps = np.full(NB, P, np.int64)
    caps[-1] = SHARD - (NB - 1) * P
    perms = []
    for cc in range(N_CORES):
        lo = cc * SHARD
        order = np.argsort(-deg[lo:lo + SHARD], kind="stable")
        heap = [(0.0, int(b)) for b in range(NB)]
        heapq.heapify(heap)
        fill = np.zeros(NB, np.int64)
        perm = np.empty(SHARD, np.int64)
        degs = deg[lo:lo + SHARD]
        for ol in order:
            while True:
                w, b = heapq.heappop(heap)
                if fill[b] < caps[b]:
                    break
            perm[ol] = b * P + fill[b]
            fill[b] += 1
            if fill[b] < caps[b]:
                heapq.heappush(heap, (w + float(degs[ol]), b))
        perms.append(perm)

    per_core = []
    tb = 0
    for cc in range(N_CORES):
        lo, hi = cc * SHARD, (cc + 1) * SHARD
        m = (alldst >= lo) & (alldst < hi)
        s = allsrc[m]
        d = perms[cc][alldst[m] - lo]       # permuted local positions
        blk = d // P
        order = np.argsort(blk, kind="stable")
        s, d, blk = s[order], d[order], blk[order]
        counts = np.bincount(blk, minlength=NB)
        tb = max(tb, int(np.ceil(counts.max() / P)))
        per_core.append((s, d, counts))
    NT = NB * tb

    dinv_pad = np.ones(NPAD, np.float32)
    dinv_pad[:N] = dinv
    xt = np.zeros((IN_F, NPAD), np.float32)
    xt[:, :N] = x.T
    w1f = np.asarray(W1, np.float32)
    cb16_common = np.concatenate([
        xt[:P].astype(bf),
        w1f[:P].astype(bf),
        np.asarray(W2, np.float32).astype(bf),
        np.asarray(Wc, np.float32).astype(bf),
        np.broadcast_to(np.arange(P, dtype=np.float32), (P, P)).astype(bf),
        np.eye(P, dtype=np.float32).astype(bf),
    ], axis=1)
    cb37 = np.concatenate([xt[P:].astype(bf), w1f[P:].astype(bf)], axis=1)
    d2 = np.ones((N_CORES, SHARD_PAD), np.float32)
    for cc in range(N_CORES):
        d2[cc, perms[cc]] = dinv[cc * SHARD:(cc + 1) * SHARD]
    cf32_common = np.concatenate([
        np.broadcast_to(np.asarray(b1, np.float32), (P, HID)),
        np.broadcast_to(np.asarray(b2, np.float32), (P, HID)),
        np.broadcast_to(np.asarray(bc, np.float32), (P, 2)),
        dinv_pad.reshape(NT1, P).T,
        d2.reshape(N_CORES * NB, P).T,
    ], axis=1).astype(np.float32)

    in_maps = []
    for cc in range(N_CORES):
        s, d, counts = per_core[cc]
        idx_u = np.zeros((NT, P), np.int64)
        dst_u = np.full((NT, P), -1.0, np.float32)
        offs = np.concatenate([[0], np.cumsum(counts)])
        for b in range(NB):
            es = s[offs[b]:offs[b + 1]]
            ed = d[offs[b]:offs[b + 1]] % P
            nloc = len(es)
            buf_s = np.zeros(tb * P, np.int64)
            buf_d = np.full(tb * P, -1.0, np.float32)
            buf_s[:nloc] = es
            buf_d[:nloc] = ed
            idx_u[b * tb:(b + 1) * tb] = buf_s.reshape(tb, P)
            dst_u[b * tb:(b + 1) * tb] = buf_d.reshape(tb, P)
        idx_u = idx_u.T.copy()
        dst_u = dst_u.T.copy()
        src_core = idx_u // SHARD
        src_newpos = np.empty_like(idx_u)
        for c2 in range(N_CORES):
            sel = src_core == c2
            src_newpos[sel] = perms[c2][idx_u[sel] % SHARD]
        idx2_u = src_core * SHARD_PAD + src_newpos
        lo = cc * SHARD
        dv = np.ones(SHARD_PAD, np.float32)
        dv[perms[cc]] = dinv[lo:lo + SHARD]
        f32_part = np.ascontiguousarray(np.concatenate(
            [cf32_common, dv.reshape(NB, P).T], axis=1).astype(np.float32))
        i32_part = np.ascontiguousarray(
            np.concatenate([idx_u, idx2_u], axis=1).astype(np.int32))
        m = {
            "cb16": np.concatenate([cb16_common, dst_u.astype(bf),
                                    f32_part.view(bf), i32_part.view(bf)],
                                   axis=1),
            "cb37": cb37,
        }
        in_maps.append(m)
    return tb, in_maps, perms


class _Runner:
    """Compile the Bass SPMD program once and execute it on cores 0-7 via
    the PJRT path (modeled on concourse.bass2jax.run_bass_via_pjrt)."""

    def __init__(self, nc, n_cores=8):
        import jax
        import concourse.mybir as mybir
        from jax.sharding import Mesh, PartitionSpec
        from jax.experimental.shard_map import shard_map
        from concourse.bass2jax import (_bass_exec_p, partition_id_tensor,
                                        install_neuronx_cc_hook)
        install_neuronx_cc_hook()
        self.jax = jax
        self.n_cores = n_cores
        in_names, out_names, out_avals = [], [], []
        partition_name = (nc.partition_id_tensor.name
                          if nc.partition_id_tensor else None)
        for alloc in nc.m.functions[0].allocations:
            if not isinstance(alloc, mybir.MemoryLocationSet):
                continue
            name = alloc.memorylocations[0].name
            if alloc.kind == "ExternalInput":
                if name != partition_name:
                    in_names.append(name)
            elif alloc.kind == "ExternalOutput":
                out_names.append(name)
                out_avals.append(jax.core.ShapedArray(
                    tuple(alloc.tensor_shape), mybir.dt.np(alloc.dtype)))
        self.in_names, self.out_names, self.out_avals = \
            in_names, out_names, out_avals
        n_params = len(in_names)
        all_in = list(in_names) + list(out_names)
        if partition_name is not None:
            all_in.append(partition_name)

        def _body(*args):
            operands = list(args)
            if partition_name is not None:
                operands.append(partition_id_tensor())
            outs = _bass_exec_p.bind(
                *operands, out_avals=tuple(out_avals), in_names=tuple(all_in),
                out_names=tuple(out_names), lowering_input_output_aliases=(),
                sim_require_finite=True, sim_require_nnan=True, nc=nc)
            return tuple(outs)

        devices = jax.devices()[:n_cores]
        self.mesh = Mesh(np.asarray(devices), ("core",))
        n_outs = len(out_avals)
        in_specs = (PartitionSpec("core"),) * (n_params + n_outs)
        out_specs = (PartitionSpec("core"),) * n_outs
        self.fn = jax.jit(
            shard_map(_body, mesh=self.mesh, in_specs=in_specs,
                      out_specs=out_specs, check_rep=False),
            keep_unused=True)

    def prep_inputs(self, in_maps):
        import jax
        from jax.sharding import NamedSharding, PartitionSpec
        concat = [np.concatenate([np.asarray(m[name]) for m in in_maps], axis=0)
                  for name in self.in_names]
        zeros = [np.zeros((self.n_cores * a.shape[0], *a.shape[1:]), a.dtype)
                 for a in self.out_avals]
        sharding = NamedSharding(self.mesh, PartitionSpec("core"))
        return [jax.device_put(a, sharding) for a in concat + zeros]

    def run(self, dev_args):
        outs = self.fn(*dev_args)
        self.jax.block_until_ready(outs)
        return outs

    def results(self, outs):
        res = []
        for c in range(self.n_cores):
            d = {}
            for i, name in enumerate(self.out_names):
                d[name] = np.asarray(outs[i]).reshape(
                    self.n_cores, *self.out_avals[i].shape)[c]
            res.append(d)
        return res


_CACHED = {}


def kernel(x, edge_index, W1, b1, W2, b2, Wc, bc):
    tb, in_maps, perms = _prepare(x, edge_index, W1, b1, W2, b2, Wc, bc)
    key = tb
    if key not in _CACHED:
        nc = _build(tb)
        _CACHED[key] = _Runner(nc)
    r = _CACHED[key]
    dev = r.prep_inputs(in_maps)
    outs = r.run(dev)
    res = r.results(outs)
    full = np.concatenate([res[c]["out"][perms[c]] for c in range(N_CORES)],
                          axis=0)
    return full.astype(np.float32)

